# revision 7
# baseline (speedup 1.0000x reference)
"""Trainium2 Bass kernel: single attention head (B=8, S=2048, E=1024, H=64).

Sharding: data-parallel over batch -- each of the 8 NeuronCores computes one
batch element's full attention. No collectives needed; every HBM byte is read
exactly once chip-wide.

Per-core pipeline (one batch element):
  - Inputs are staged host-side as X^T ([E, S], contiguous) so the contraction
    dim lands on SBUF partitions with perfectly contiguous DMA.
  - fp16 compute: X chunks are cast f32->f16 during the SWDGE DMA.
  - Projections q^T/k^T/v^T = W^T @ X^T on TensorE, PSUM-accumulated over 8
    K-chunks of 128. bq folds into q^T during PSUM evacuation; bk cancels in
    softmax (adds a per-query constant to every score); bv folds into v.
  - DMA order: q-left, K, q-right, v-left, v-right (halves = 1024 of the 2048
    sequence columns). The left half of the transposed score matrix needs only
    q-left + K, so ScalarE's exp chain -- the serial ~37us of this kernel --
    starts as soon as K is resident, ~25us earlier than a full-Q schedule.
  - Scores are computed TRANSPOSED: S^T[sk, sq] = k^T.T @ q^T, so softmax's
    sum runs over the partition axis, which we get for free by augmenting v
    with a ones column: [v | 1].T @ exp(S^T) yields [out^T ; rowsums].
  - exp on ScalarE (fp32 PSUM in -> fp16 SBUF out), scale=1/8 fused.
  - Late-arriving projection matmuls (q-right, v) are interleaved into the
    scores stream so the PE FIFO never stalls the exp chain.
  - AV accumulates into two [65, 1024] PSUM tiles that slot-chain through the
    same 2-bank pool the q-right/v projections used (PSUM stays at 8 banks).
  - Finalize: transpose 128-column chunks via TensorE, divide by the rowsum
    column with VectorE reciprocal + tensor_scalar, batched fp32 DMAs out.
"""

import numpy as np

import concourse.bass as bass  # noqa: F401  (engine namespaces live on nc)
import concourse.mybir as mybir
import concourse.tile as tile
from concourse import bacc
from concourse.bass_utils import run_bass_kernel_spmd
from concourse.masks import make_identity

B, S, E, H = 8, 2048, 1024, 64
EC = E // 128   # contraction chunks per projection
NT = S // 128   # key tiles
HS = S // 2     # half sequence
F16 = mybir.dt.float16
F32 = mybir.dt.float32

_CACHE = {}


def _build_nc():
    nc = bacc.Bacc(None)
    xq = nc.declare_dram_parameter("xqt", [E, S], F32, isOutput=False)
    xk = nc.declare_dram_parameter("xkt", [E, S], F32, isOutput=False)
    xv = nc.declare_dram_parameter("xvt", [E, S], F32, isOutput=False)
    wq = nc.declare_dram_parameter("wq", [E, H], F32, isOutput=False)
    wk = nc.declare_dram_parameter("wk", [E, H], F32, isOutput=False)
    wv = nc.declare_dram_parameter("wv", [E, H], F32, isOutput=False)
    bq = nc.declare_dram_parameter("bq", [H, 1], F32, isOutput=False)
    bv = nc.declare_dram_parameter("bv", [H, 1], F32, isOutput=False)
    out = nc.declare_dram_parameter("out", [S, H], F32, isOutput=True)

    Exp = mybir.ActivationFunctionType.Exp

    with tile.TileContext(nc) as tc:
        with tc.tile_pool(name="const", bufs=1) as const, \
             tc.tile_pool(name="xio", bufs=6) as xio, \
             tc.tile_pool(name="xioh", bufs=6) as xioh, \
             tc.tile_pool(name="ptp", bufs=NT) as ptp, \
             tc.tile_pool(name="p5sb", bufs=2) as p5sb:

            # weights: one casting SWDGE DMA each, at the head of the queue
            wts = {}
            for nm, dram in (("q", wq), ("k", wk), ("v", wv)):
                wt = const.tile([128, EC, H], F16, name=f"w{nm}")
                nc.gpsimd.dma_start(
                    out=wt[:], in_=dram[:].rearrange("(c p) h -> p c h", p=128))
                wts[nm] = wt
            bq_t = const.tile([H, 1], F32, name="bq_t")
            nc.sync.dma_start(out=bq_t[:], in_=bq[:])
            bv_t = const.tile([H, 1], F32, name="bv_t")
            nc.sync.dma_start(out=bv_t[:], in_=bv[:])

            qt = const.tile([64, S], F16, name="qt")
            kt = const.tile([64, S], F16, name="kt")
            vt = const.tile([64, S], F16, name="vt")
            vaug = const.tile([128, NT, 80], F16, name="vaug")
            oasb = const.tile([65, S], F16, name="oasb")
            ident = const.tile([128, 128], F16, name="ident")
            osb_all = const.tile([128, NT, H], F32, name="osb_all")

            # HAM warmup: ~7us of junk matmuls so the PE clock is at 2.4GHz
            # before the first projection matmul issues.
            wsrc = const.tile([64, 512], F16, name="wsrc")
            nc.vector.memset(wsrc[:], 0.0)
            with tc.tile_pool(name="wps", bufs=1, space="PSUM") as wps:
                wpsum = wps.tile([64, 512], F32, name="wpsum")
                for _ in range(24):
                    nc.tensor.matmul(wpsum[:], wsrc[:, 0:64], wsrc[:],
                                     start=True, stop=True)

            def half_chunk(nm, xdram, ps_ap, half, c):
                # one 128-row contraction chunk of a half-column projection;
                # ps_ap is a [64, 1024] PSUM view
                lo = half * HS
                xt_ = xioh.tile([128, HS], F16, tag="xth", name=f"x{nm}{half}{c}")
                nc.gpsimd.dma_start(
                    out=xt_[:], in_=xdram[c * 128:(c + 1) * 128, lo:lo + HS])
                for n in range(2):
                    nc.tensor.matmul(
                        ps_ap[:, n * 512:(n + 1) * 512],
                        wts[nm][:, c, :], xt_[:, n * 512:(n + 1) * 512],
                        start=(c == 0), stop=(c == EC - 1))

            # q-left + k projections (PSUM: 2 + 4 banks)
            with tc.tile_pool(name="ppsum", bufs=1, space="PSUM") as pp:
                psqL = pp.tile([64, HS], F32, tag="qL", name="psqL")
                for c in range(EC):
                    half_chunk("q", xq, psqL[:], 0, c)
                nc.vector.tensor_scalar_add(qt[:, 0:HS], psqL[:], bq_t[:])
                psk = pp.tile([64, S], F32, tag="k", name="psk")
                for c in range(EC):
                    xt_ = xio.tile([128, S], F16, tag="xt", name=f"xk{c}")
                    nc.gpsimd.dma_start(out=xt_[:], in_=xk[c * 128:(c + 1) * 128, :])
                    for n in range(S // 512):
                        nc.tensor.matmul(
                            psk[:, n * 512:(n + 1) * 512],
                            wts["k"][:, c, :], xt_[:, n * 512:(n + 1) * 512],
                            start=(c == 0), stop=(c == EC - 1))
                nc.vector.tensor_copy(kt[:], psk[:])

                # attention phase (PSUM: ppsum still open but its banks are
                # not reused until it closes -- so keep the remaining pools at
                # 8 total: big 2x2 + scores 2x2. ppsum closes right here.)

            with tc.tile_pool(name="big", bufs=2, space="PSUM") as big, \
                 tc.tile_pool(name="spsum", bufs=2, space="PSUM") as sps:

                def s_half(t, h2):
                    st = sps.tile([128, HS], F32, tag="st", name=f"st{t}_{h2}")
                    for n in range(2):
                        nc.tensor.matmul(
                            st[:, n * 512:(n + 1) * 512],
                            kt[:, t * 128:(t + 1) * 128],
                            qt[:, h2 * HS + n * 512: h2 * HS + (n + 1) * 512],
                            start=True, stop=True)
                    nc.scalar.activation(
                        pts[t][:, h2 * HS:(h2 + 1) * HS], st[:], Exp, scale=0.125)

                pts = [ptp.tile([128, S], F16, tag="pt", name=f"pt{t}")
                       for t in range(NT)]

                # left scores + exp, with q-right projection interleaved
                psqR = big.tile([65, HS], F32, tag="big", name="psqR")
                s_half(0, 0)
                s_half(1, 0)
                for c in range(EC):
                    half_chunk("q", xq, psqR[0:64, :], 1, c)
                    s_half(2 + c, 0)
                for t in range(10, NT):
                    s_half(t, 0)
                nc.vector.tensor_scalar_add(qt[:, HS:S], psqR[0:64, :], bq_t[:])

                # right scores + exp, with v projections interleaved
                vpsL = big.tile([65, HS], F32, tag="big", name="vpsL")
                for c in range(EC):
                    half_chunk("v", xv, vpsL[0:64, :], 0, c)
                    s_half(c, 1)
                nc.vector.tensor_scalar_add(vt[:, 0:HS], vpsL[0:64, :], bv_t[:])
                nc.vector.memset(vaug[:, :, 64], 1.0)
                nc.sync.dma_start_transpose(vaug[:, 0:NT // 2, 0:64], vt[:, 0:HS])
                make_identity(nc, ident[:])  # late: keeps GpSimd queue clear

                vpsR = big.tile([65, HS], F32, tag="big", name="vpsR")
                for c in range(EC):
                    half_chunk("v", xv, vpsR[0:64, :], 1, c)
                    s_half(8 + c, 1)
                nc.vector.tensor_scalar_add(vt[:, HS:S], vpsR[0:64, :], bv_t[:])
                nc.sync.dma_start_transpose(vaug[:, NT // 2:NT, 0:64], vt[:, HS:S])

                oaL = big.tile([65, HS], F32, tag="big", name="oaL")
                oaR = big.tile([65, HS], F32, tag="big", name="oaR")

                def av_group(ts, cqs):
                    for t in ts:
                        for cq in cqs:
                            tgt = oaL if cq < 2 else oaR
                            nc.tensor.matmul(
                                tgt[:, (cq % 2) * 512:(cq % 2 + 1) * 512],
                                vaug[:, t, 0:65],
                                pts[t][:, cq * 512:(cq + 1) * 512],
                                start=(t == 0), stop=(t == NT - 1),
                                skip_group_check=True)

                av_group(range(0, 8), (0, 1))
                av_group(range(0, 8), (2, 3))
                av_group(range(8, NT), (0, 1))
                av_group(range(8, NT), (2, 3))
                nc.vector.tensor_copy(oasb[:, 0:HS], oaL[:])
                nc.vector.tensor_copy(oasb[:, HS:S], oaR[:])

            # finalize: transpose, normalize, store (PSUM: 2 x 1 bank)
            out_r = out[:].rearrange("(t p) h -> p t h", p=128)
            with tc.tile_pool(name="p5ps", bufs=2, space="PSUM") as p5ps:
                for cq in range(4):
                    for jj in range(4):
                        j = cq * 4 + jj
                        tr = p5ps.tile([128, 65], F16, tag="tr", name=f"tr{j}")
                        nc.tensor.transpose(
                            tr[:], oasb[:, j * 128:(j + 1) * 128], ident[0:65, 0:65])
                        rc = p5sb.tile([128, 1], F32, tag="rc", name=f"rc{j}")
                        nc.vector.reciprocal(rc[:], tr[:, 64:65])
                        nc.vector.tensor_scalar(
                            osb_all[:, j, :], tr[:, 0:64], rc[:], None,
                            op0=mybir.AluOpType.mult)
                    nc.sync.dma_start(
                        out=out_r[:, cq * 4:(cq + 1) * 4, :],
                        in_=osb_all[:, cq * 4:(cq + 1) * 4, :])

    nc.finalize()
    return nc


def get_nc():
    if "nc" not in _CACHE:
        _CACHE["nc"] = _build_nc()
    return _CACHE["nc"]


def make_in_maps(inputs):
    q = np.asarray(inputs["query"], np.float32)
    k = np.asarray(inputs["key_"], np.float32)
    v = np.asarray(inputs["value"], np.float32)
    wq = np.ascontiguousarray(np.asarray(inputs["Wq"], np.float32))
    wk = np.ascontiguousarray(np.asarray(inputs["Wk"], np.float32))
    wv = np.ascontiguousarray(np.asarray(inputs["Wv"], np.float32))
    bq = np.ascontiguousarray(np.asarray(inputs["bq"], np.float32).reshape(H, 1))
    bv = np.ascontiguousarray(np.asarray(inputs["bv"], np.float32).reshape(H, 1))
    in_maps = []
    for b in range(B):
        in_maps.append({
            "xqt": np.ascontiguousarray(q[b].T),
            "xkt": np.ascontiguousarray(k[b].T),
            "xvt": np.ascontiguousarray(v[b].T),
            "wq": wq, "wk": wk, "wv": wv,
            "bq": bq, "bv": bv,
        })
    return in_maps


def kernel(**inputs):
    nc = get_nc()
    in_maps = make_in_maps(inputs)
    res = run_bass_kernel_spmd(nc, in_maps, list(range(B)))
    return np.stack([res.results[b]["out"] for b in range(B)], axis=0)


# revision 8
# speedup vs baseline: 1.1713x; 1.1713x over previous
"""Trainium2 Bass kernel: single attention head (B=8, S=2048, E=1024, H=64).

Sharding: data-parallel over batch -- each of the 8 NeuronCores computes one
batch element's full attention. No collectives needed; every HBM byte is read
exactly once chip-wide.

Per-core pipeline (one batch element):
  - Inputs are staged host-side as X^T ([E, S], contiguous) so the contraction
    dim lands on SBUF partitions with perfectly contiguous DMA.
  - fp16 compute: X chunks are cast f32->f16 during the SWDGE DMA.
  - Projections q^T/k^T/v^T = W^T @ X^T on TensorE, PSUM-accumulated over 8
    K-chunks of 128. bq folds into q^T during PSUM evacuation; bk cancels in
    softmax (adds a per-query constant to every score); bv folds into v.
  - DMA order: q-left, K, q-right, v-left, v-right (halves = 1024 of the 2048
    sequence columns). The left half of the transposed score matrix needs only
    q-left + K, so ScalarE's exp chain -- the serial ~37us of this kernel --
    starts as soon as K is resident, ~25us earlier than a full-Q schedule.
  - Scores are computed TRANSPOSED: S^T[sk, sq] = k^T.T @ q^T, so softmax's
    sum runs over the partition axis, which we get for free by augmenting v
    with a ones column: [v | 1].T @ exp(S^T) yields [out^T ; rowsums].
  - exp on ScalarE (fp32 PSUM in -> fp16 SBUF out), scale=1/8 fused.
  - Late-arriving projection matmuls (q-right, v) are interleaved into the
    scores stream so the PE FIFO never stalls the exp chain.
  - AV accumulates into two [65, 1024] PSUM tiles that slot-chain through the
    same 2-bank pool the q-right/v projections used (PSUM stays at 8 banks).
  - Finalize: transpose 128-column chunks via TensorE, divide by the rowsum
    column with VectorE reciprocal + tensor_scalar, batched fp32 DMAs out.
"""

import numpy as np

import concourse.bass as bass  # noqa: F401  (engine namespaces live on nc)
import concourse.mybir as mybir
import concourse.tile as tile
from concourse import bacc
from concourse.bass_utils import run_bass_kernel_spmd
from concourse.masks import make_identity

B, S, E, H = 8, 2048, 1024, 64
EC = E // 128   # contraction chunks per projection
NT = S // 128   # key tiles
HS = S // 2     # half sequence
F16 = mybir.dt.float16
F32 = mybir.dt.float32

_CACHE = {}


def _build_nc():
    nc = bacc.Bacc(None)
    xqh = [nc.declare_dram_parameter(f"xqt{h}", [E, HS], F32, isOutput=False)
           for h in range(2)]
    xk = nc.declare_dram_parameter("xkt", [E, S], F32, isOutput=False)
    xvh = [nc.declare_dram_parameter(f"xvt{h}", [E, HS], F32, isOutput=False)
           for h in range(2)]
    wq = nc.declare_dram_parameter("wq", [E, H], F32, isOutput=False)
    wk = nc.declare_dram_parameter("wk", [E, H], F32, isOutput=False)
    wv = nc.declare_dram_parameter("wv", [E, H], F32, isOutput=False)
    bq = nc.declare_dram_parameter("bq", [H, 1], F32, isOutput=False)
    bv = nc.declare_dram_parameter("bv", [H, 1], F32, isOutput=False)
    out = nc.declare_dram_parameter("out", [S, H], F32, isOutput=True)

    Exp = mybir.ActivationFunctionType.Exp

    with tile.TileContext(nc) as tc:
        with tc.tile_pool(name="const", bufs=1) as const, \
             tc.tile_pool(name="xio", bufs=6) as xio, \
             tc.tile_pool(name="xioh", bufs=6) as xioh, \
             tc.tile_pool(name="ptp", bufs=NT) as ptp, \
             tc.tile_pool(name="p5sb", bufs=2) as p5sb:

            # weights: one casting SWDGE DMA each, at the head of the queue
            wts = {}
            for nm, dram in (("q", wq), ("k", wk), ("v", wv)):
                wt = const.tile([128, EC, H], F16, name=f"w{nm}")
                nc.gpsimd.dma_start(
                    out=wt[:], in_=dram[:].rearrange("(c p) h -> p c h", p=128))
                wts[nm] = wt
            bq_t = const.tile([H, 1], F32, name="bq_t")
            nc.sync.dma_start(out=bq_t[:], in_=bq[:])
            bv_t = const.tile([H, 1], F32, name="bv_t")
            nc.sync.dma_start(out=bv_t[:], in_=bv[:])

            qt = const.tile([64, S], F16, name="qt")
            kt = const.tile([64, S], F16, name="kt")
            vt = const.tile([64, S], F16, name="vt")
            vaug = const.tile([128, NT, 80], F16, name="vaug")
            oasb = const.tile([65, S], F16, name="oasb")
            ident = const.tile([128, 128], F16, name="ident")
            osb_all = const.tile([128, NT, H], F32, name="osb_all")

            # HAM warmup: ~7us of junk matmuls so the PE clock is at 2.4GHz
            # before the first projection matmul issues.
            wsrc = const.tile([64, 512], F16, name="wsrc")
            nc.vector.memset(wsrc[:], 0.0)
            with tc.tile_pool(name="wps", bufs=1, space="PSUM") as wps:
                wpsum = wps.tile([64, 512], F32, name="wpsum")
                for _ in range(24):
                    nc.tensor.matmul(wpsum[:], wsrc[:, 0:64], wsrc[:],
                                     start=True, stop=True)

            def half_chunk(nm, xdrams, ps_ap, half, c):
                # one 128-row contraction chunk of a half-column projection;
                # ps_ap is a [64, 1024] PSUM view. The halves are separate
                # contiguous DRAM tensors (host-staged) so each chunk DMA is
                # one fully-contiguous 512KB read.
                xt_ = xioh.tile([128, HS], F16, tag="xth", name=f"x{nm}{half}{c}")
                nc.gpsimd.dma_start(
                    out=xt_[:], in_=xdrams[half][c * 128:(c + 1) * 128, :])
                for n in range(2):
                    nc.tensor.matmul(
                        ps_ap[:, n * 512:(n + 1) * 512],
                        wts[nm][:, c, :], xt_[:, n * 512:(n + 1) * 512],
                        start=(c == 0), stop=(c == EC - 1))

            # q-left + k projections (PSUM: 2 + 4 banks)
            with tc.tile_pool(name="ppsum", bufs=1, space="PSUM") as pp:
                psqL = pp.tile([64, HS], F32, tag="qL", name="psqL")
                for c in range(EC):
                    half_chunk("q", xqh, psqL[:], 0, c)
                nc.vector.tensor_scalar_add(qt[:, 0:HS], psqL[:], bq_t[:])
                psk = pp.tile([64, S], F32, tag="k", name="psk")
                for c in range(EC):
                    xt_ = xio.tile([128, S], F16, tag="xt", name=f"xk{c}")
                    nc.gpsimd.dma_start(out=xt_[:], in_=xk[c * 128:(c + 1) * 128, :])
                    for n in range(S // 512):
                        nc.tensor.matmul(
                            psk[:, n * 512:(n + 1) * 512],
                            wts["k"][:, c, :], xt_[:, n * 512:(n + 1) * 512],
                            start=(c == 0), stop=(c == EC - 1))
                nc.vector.tensor_copy(kt[:], psk[:])

                # attention phase (PSUM: ppsum still open but its banks are
                # not reused until it closes -- so keep the remaining pools at
                # 8 total: big 2x2 + scores 2x2. ppsum closes right here.)

            with tc.tile_pool(name="big", bufs=2, space="PSUM") as big, \
                 tc.tile_pool(name="spsum", bufs=2, space="PSUM") as sps:

                def s_half(t, h2):
                    st = sps.tile([128, HS], F32, tag="st", name=f"st{t}_{h2}")
                    for n in range(2):
                        nc.tensor.matmul(
                            st[:, n * 512:(n + 1) * 512],
                            kt[:, t * 128:(t + 1) * 128],
                            qt[:, h2 * HS + n * 512: h2 * HS + (n + 1) * 512],
                            start=True, stop=True)
                    nc.scalar.activation(
                        pts[t][:, h2 * HS:(h2 + 1) * HS], st[:], Exp, scale=0.125)

                pts = [ptp.tile([128, S], F16, tag="pt", name=f"pt{t}")
                       for t in range(NT)]

                # left scores + exp, with q-right projection interleaved
                psqR = big.tile([65, HS], F32, tag="big", name="psqR")
                s_half(0, 0)
                s_half(1, 0)
                for c in range(EC):
                    half_chunk("q", xqh, psqR[0:64, :], 1, c)
                    s_half(2 + c, 0)
                for t in range(10, NT):
                    s_half(t, 0)
                nc.vector.tensor_scalar_add(qt[:, HS:S], psqR[0:64, :], bq_t[:])

                # right scores + exp, with v projections interleaved
                vpsL = big.tile([65, HS], F32, tag="big", name="vpsL")
                for c in range(EC):
                    half_chunk("v", xvh, vpsL[0:64, :], 0, c)
                    s_half(c, 1)
                nc.vector.tensor_scalar_add(vt[:, 0:HS], vpsL[0:64, :], bv_t[:])
                nc.vector.memset(vaug[:, :, 64], 1.0)
                nc.sync.dma_start_transpose(vaug[:, 0:NT // 2, 0:64], vt[:, 0:HS])
                make_identity(nc, ident[:])  # late: keeps GpSimd queue clear

                vpsR = big.tile([65, HS], F32, tag="big", name="vpsR")
                for c in range(EC):
                    half_chunk("v", xvh, vpsR[0:64, :], 1, c)
                    s_half(8 + c, 1)
                nc.vector.tensor_scalar_add(vt[:, HS:S], vpsR[0:64, :], bv_t[:])
                nc.sync.dma_start_transpose(vaug[:, NT // 2:NT, 0:64], vt[:, HS:S])

                oaL = big.tile([65, HS], F32, tag="big", name="oaL")
                oaR = big.tile([65, HS], F32, tag="big", name="oaR")

                def av_group(ts, cqs):
                    for t in ts:
                        for cq in cqs:
                            tgt = oaL if cq < 2 else oaR
                            nc.tensor.matmul(
                                tgt[:, (cq % 2) * 512:(cq % 2 + 1) * 512],
                                vaug[:, t, 0:65],
                                pts[t][:, cq * 512:(cq + 1) * 512],
                                start=(t == 0), stop=(t == NT - 1),
                                skip_group_check=True)

                av_group(range(0, 8), (0, 1))
                av_group(range(0, 8), (2, 3))
                av_group(range(8, NT), (0, 1))
                av_group(range(8, NT), (2, 3))
                nc.vector.tensor_copy(oasb[:, 0:HS], oaL[:])
                nc.vector.tensor_copy(oasb[:, HS:S], oaR[:])

            # finalize: transpose, normalize, store (PSUM: 2 x 1 bank)
            out_r = out[:].rearrange("(t p) h -> p t h", p=128)
            with tc.tile_pool(name="p5ps", bufs=2, space="PSUM") as p5ps:
                for cq in range(4):
                    for jj in range(4):
                        j = cq * 4 + jj
                        tr = p5ps.tile([128, 65], F16, tag="tr", name=f"tr{j}")
                        nc.tensor.transpose(
                            tr[:], oasb[:, j * 128:(j + 1) * 128], ident[0:65, 0:65])
                        rc = p5sb.tile([128, 1], F32, tag="rc", name=f"rc{j}")
                        nc.vector.reciprocal(rc[:], tr[:, 64:65])
                        nc.vector.tensor_scalar(
                            osb_all[:, j, :], tr[:, 0:64], rc[:], None,
                            op0=mybir.AluOpType.mult)
                    nc.sync.dma_start(
                        out=out_r[:, cq * 4:(cq + 1) * 4, :],
                        in_=osb_all[:, cq * 4:(cq + 1) * 4, :])

    nc.finalize()
    return nc


def get_nc():
    if "nc" not in _CACHE:
        _CACHE["nc"] = _build_nc()
    return _CACHE["nc"]


def make_in_maps(inputs):
    q = np.asarray(inputs["query"], np.float32)
    k = np.asarray(inputs["key_"], np.float32)
    v = np.asarray(inputs["value"], np.float32)
    wq = np.ascontiguousarray(np.asarray(inputs["Wq"], np.float32))
    wk = np.ascontiguousarray(np.asarray(inputs["Wk"], np.float32))
    wv = np.ascontiguousarray(np.asarray(inputs["Wv"], np.float32))
    bq = np.ascontiguousarray(np.asarray(inputs["bq"], np.float32).reshape(H, 1))
    bv = np.ascontiguousarray(np.asarray(inputs["bv"], np.float32).reshape(H, 1))
    in_maps = []
    for b in range(B):
        qT, vT = q[b].T, v[b].T
        in_maps.append({
            "xqt0": np.ascontiguousarray(qT[:, 0:HS]),
            "xqt1": np.ascontiguousarray(qT[:, HS:S]),
            "xkt": np.ascontiguousarray(k[b].T),
            "xvt0": np.ascontiguousarray(vT[:, 0:HS]),
            "xvt1": np.ascontiguousarray(vT[:, HS:S]),
            "wq": wq, "wk": wk, "wv": wv,
            "bq": bq, "bv": bv,
        })
    return in_maps


def kernel(**inputs):
    nc = get_nc()
    in_maps = make_in_maps(inputs)
    res = run_bass_kernel_spmd(nc, in_maps, list(range(B)))
    return np.stack([res.results[b]["out"] for b in range(B)], axis=0)
